# revision 14
# baseline (speedup 1.0000x reference)
"""Trainium2 Bass kernel for a GNN message-passing layer.

Reference computation (per node n, neighbors k=0..31):
  sa = src_atom_emb[atomic]            [N,128]
  ta = tgt_atom_emb[atomic]            [N,128]
  sd = silu(nde @ src_dir_W + b)       [N,64]
  td = silu(nde @ tgt_dir_W + b)       [N,64]
  edist = silu(ede @ dist_W + b)       [N,K,128]
  feat  = [edist | sd[nbr] | sa[nbr] | td | ta]   [N,K,512]
  out   = sum_k(mask*feat) / (sum_k mask + 1e-5)  [N,512]

Strategy (8 cores, nodes sharded 1250/core, SPMD, no collectives, and
no SWDGE gather -- the per-row descriptor cost dominated the previous
design):
  - dist branch: host premasks ede (silu(0)=0 for 0/1 masks), casts to
    fp16, and lays it out k-plane-major [128 fin, nb, k, 128 nodes].
    PE streams it against dist_W split into fp16 hi+lo halves (two
    accumulating matmuls recover near-fp32 precision; plain bf16/fp16
    weights fail the 2e-2 rel bar because ~2% of outputs sit below the
    rel-err denominator floor).  SiLU on ACT in fp32, then the k-sum is
    5 fully-contiguous pairwise tree adds on DVE (the strided
    tensor_reduce form runs ~5x slower).
  - sender-dir sums: the host dedupes each 128-node group's neighbor
    set (max 1943 unique, padded to 2048 slots), ships the gathered nde
    rows (fp16) plus an fp16 multiplicity matrix A[slot, dst].  sd is
    built on-PE (fp16, exact-count A), and the neighbor sum is 16
    accumulating [128x128]x[128,64] matmuls per group.
  - sender-atom sums: only 100 distinct embedding rows exist, so the
    host ships a per-node histogram over atom ids (fp16, exact counts)
    and the sum is one matmul per group against the fp16 embedding.
  - recv features: td via fp32 matmuls (per-node values get no
    mask-mean averaging, so they need full precision) + PE transpose;
    ta via one-hot matmul.  Scales cnt/(cnt+eps), 1/(cnt+eps) come
    precomputed from host.
"""

import os
import sys

import numpy as np

sys.path.insert(0, "/opt/trn_rl_repo")

import concourse.bacc as bacc  # noqa: E402
import concourse.bass as bass  # noqa: E402,F401
import concourse.mybir as mybir  # noqa: E402
import concourse.tile as tile  # noqa: E402
from concourse.bass_utils import run_bass_kernel_spmd  # noqa: E402

# Problem shape (hardcoded; harness always uses these).
N_CORES = 8
N = 10000
K = 32
NLOC = N // N_CORES          # 1250 nodes per core
NPAD = 1280                  # padded to 10 groups of 128
NG = NPAD // 128             # 10 node groups
NB = NG                      # dist branch node blocks (128 nodes each)
D_DIR_IN = 10
D_DIR = 64
D_ATOM = 128
D_DIST_IN = 128
D_DIST = 128
NUM_ELEM = 100
SLOTS = 2048                 # deduped neighbor slots per group
SCH = SLOTS // 128           # 16 slot chunks per group
FP32 = mybir.dt.float32
FP16 = mybir.dt.float16

_CACHED = {}


def _build_program():
    nc = bacc.Bacc(
        "TRN2",
        target_bir_lowering=False,
        debug=False,
        enable_asserts=False,
        num_devices=N_CORES,
    )

    # per-core inputs
    edeT = nc.dram_tensor("edeT", [128, NB * K * 128], FP16, kind="ExternalInput")
    a_mat = nc.dram_tensor("a_mat", [128, NG * SLOTS], FP16, kind="ExternalInput")
    ndeL = nc.dram_tensor("ndeL", [D_DIR_IN + 1, NG * SLOTS], FP16,
                          kind="ExternalInput")
    histT = nc.dram_tensor("histT", [128, NPAD], FP16, kind="ExternalInput")
    ohT = nc.dram_tensor("ohT", [128, NPAD], FP16, kind="ExternalInput")
    ndeTl = nc.dram_tensor("ndeTl", [D_DIR_IN + 1, NPAD], FP32, kind="ExternalInput")
    scl = nc.dram_tensor("scl", [128, 2 * NG], FP32, kind="ExternalInput")
    # shared (replicated) parameters
    wdist2 = nc.dram_tensor("wdist2", [D_DIST_IN, 2 * D_DIST], FP16,
                            kind="ExternalInput")
    w_sd = nc.dram_tensor("w_sd", [D_DIR_IN + 1, D_DIR], FP16, kind="ExternalInput")
    w_td = nc.dram_tensor("w_td", [D_DIR_IN + 1, D_DIR], FP32, kind="ExternalInput")
    emb_s = nc.dram_tensor("emb_s", [128, D_ATOM], FP16, kind="ExternalInput")
    emb_t = nc.dram_tensor("emb_t", [128, D_ATOM], FP16, kind="ExternalInput")
    ident = nc.dram_tensor("ident", [128, 128], FP32, kind="ExternalInput")

    # fp16 output: rounding error is proportional to the value (<=2^-11 rel),
    # so it can never violate the floored rel-err metric; halves output DMA.
    out_d = nc.dram_tensor("out", [NLOC, 512], FP16, kind="ExternalOutput")

    Silu = mybir.ActivationFunctionType.Silu

    with tile.TileContext(nc) as tc:
        from contextlib import ExitStack

        with ExitStack() as ctx:
            const = ctx.enter_context(tc.tile_pool(name="const", bufs=1))
            ede_pool = ctx.enter_context(tc.tile_pool(name="ede_pool", bufs=8))
            silu_pool = ctx.enter_context(tc.tile_pool(name="silu_pool", bufs=2))
            t1_pool = ctx.enter_context(tc.tile_pool(name="t1_pool", bufs=3))
            t2_pool = ctx.enter_context(tc.tile_pool(name="t2_pool", bufs=2))
            t34_pool = ctx.enter_context(tc.tile_pool(name="t34_pool", bufs=2))
            ndeL_pool = ctx.enter_context(tc.tile_pool(name="ndeL_pool", bufs=4))
            a_pool = ctx.enter_context(tc.tile_pool(name="a_pool", bufs=6))
            out_pool = ctx.enter_context(tc.tile_pool(name="out_pool", bufs=2))
            ps_grp = ctx.enter_context(
                tc.tile_pool(name="ps_grp", bufs=2, space="PSUM")
            )

            # --- kick weights, then the first ede block in 4 pieces ---
            wdist2_s = const.tile([D_DIST_IN, 2 * D_DIST], FP16)
            nc.sync.dma_start(wdist2_s[:], wdist2[:, :])
            # ede streams in 2048-col half-blocks (finer arrival granularity)
            ede_tiles = {}
            for h in range(7):
                ede_tiles[h] = ede_pool.tile([128, 2048], FP16, tag="ede",
                                             name="ede_t")
                nc.sync.dma_start(
                    ede_tiles[h][:], edeT[:, h * 2048 : (h + 1) * 2048]
                )
            w_sd_s = const.tile([D_DIR_IN + 1, D_DIR], FP16)
            nc.sync.dma_start(w_sd_s[:], w_sd[:, :])
            w_td_s = const.tile([D_DIR_IN + 1, D_DIR], FP32)
            nc.sync.dma_start(w_td_s[:], w_td[:, :])
            emb_s_s = const.tile([128, D_ATOM], FP16)
            nc.sync.dma_start(emb_s_s[:], emb_s[:, :])
            emb_t_s = const.tile([128, D_ATOM], FP16)
            nc.sync.dma_start(emb_t_s[:], emb_t[:, :])
            ident_s = const.tile([128, 128], FP32)
            nc.sync.dma_start(ident_s[:], ident[:, :])
            scl_s = const.tile([128, 2 * NG], FP32)
            nc.sync.dma_start(scl_s[:], scl[:, :])
            histT_s = const.tile([128, NPAD], FP16)
            nc.sync.dma_start(histT_s[:], histT[:, :])
            ohT_s = const.tile([128, NPAD], FP16)
            nc.sync.dma_start(ohT_s[:], ohT[:, :])
            ndeTl_s = const.tile([D_DIR_IN + 1, NPAD], FP32)
            nc.sync.dma_start(ndeTl_s[:], ndeTl[:, :])
            ndeL_tiles = {}
            a_tiles = {}

            dist_acc = const.tile([128, NPAD], FP32)
            sdl_all = const.tile([128, NG * SLOTS // 2], FP16)  # [128, g, 16, 64]
            td_allT = const.tile([D_DIR, NPAD], FP32)

            # --- Phase D: dist branch (k-plane matmuls + silu + tree sums) ---
            with ExitStack() as pd_ctx:
                ps_dist = pd_ctx.enter_context(
                    tc.tile_pool(name="ps_dist", bufs=2, space="PSUM")
                )
                for nb in range(NB):
                    halves = [ede_tiles.pop(2 * nb), ede_tiles.pop(2 * nb + 1)]
                    s_tile = silu_pool.tile([128, K * 128], FP32, tag="silu")
                    for q in range(4):  # 4 psum tiles x 1024 cols
                        pd = ps_dist.tile([128, 1024], FP32, tag="pd")
                        t_half = halves[q // 2]
                        for r in range(2):  # two 512-col regions
                            lo = (q % 2) * 1024 + r * 512
                            src_ap = t_half[:, lo : lo + 512]
                            nc.tensor.matmul(
                                pd[:, r * 512 : (r + 1) * 512],
                                wdist2_s[:, 0:D_DIST],
                                src_ap,
                                start=True,
                                stop=False,
                            )
                            nc.tensor.matmul(
                                pd[:, r * 512 : (r + 1) * 512],
                                wdist2_s[:, D_DIST : 2 * D_DIST],
                                src_ap,
                                start=False,
                                stop=True,
                            )
                        nc.scalar.activation(
                            s_tile[:, q * 1024 : (q + 1) * 1024], pd[:], Silu
                        )
                    # prefetch: ndeL (tiny) plus the next two ede half-blocks
                    if nb < 4:
                        ndeL_tiles[nb] = ndeL_pool.tile(
                            [D_DIR_IN + 1, SLOTS], FP16, tag="ndeL", name="ndeL_t"
                        )
                        nc.sync.dma_start(
                            ndeL_tiles[nb][:],
                            ndeL[:, nb * SLOTS : (nb + 1) * SLOTS],
                        )
                    for h in (2 * nb + 7, 2 * nb + 8):
                        if h < 2 * NB:
                            ede_tiles[h] = ede_pool.tile(
                                [128, 2048], FP16, tag="ede", name="ede_t"
                            )
                            nc.sync.dma_start(
                                ede_tiles[h][:],
                                edeT[:, h * 2048 : (h + 1) * 2048],
                            )
                    # pairwise tree sum over the 32 k-planes (all contiguous).
                    # Pool (slow sw add) takes half of r1; DVE does the rest.
                    t1 = t1_pool.tile([128, 2048], FP32, tag="t1")
                    nc.gpsimd.tensor_add(
                        t1[:, 0:1024], s_tile[:, 0:1024], s_tile[:, 2048:3072]
                    )
                    nc.vector.tensor_add(
                        t1[:, 1024:2048], s_tile[:, 1024:2048], s_tile[:, 3072:4096]
                    )
                    t2 = t2_pool.tile([128, 1024], FP32, tag="t2")
                    nc.vector.tensor_add(t2[:], t1[:, 0:1024], t1[:, 1024:2048])
                    t3 = t34_pool.tile([128, 512], FP32, tag="t3")
                    nc.vector.tensor_add(t3[:], t2[:, 0:512], t2[:, 512:1024])
                    t4 = t34_pool.tile([128, 256], FP32, tag="t4")
                    nc.vector.tensor_add(t4[:], t3[:, 0:256], t3[:, 256:512])
                    nc.vector.tensor_add(
                        dist_acc[:, nb * 128 : (nb + 1) * 128],
                        t4[:, 0:128],
                        t4[:, 128:256],
                    )

            # --- td for all local nodes: [64, NPAD] feature-major (fp32) ---
            sg_ctx = ctx.enter_context(ExitStack())
            ps_sdl = sg_ctx.enter_context(
                tc.tile_pool(name="ps_sdl", bufs=2, space="PSUM")
            )
            for i, (lo, w) in enumerate([(0, 512), (512, 512), (1024, 256)]):
                ps_td = ps_sdl.tile([D_DIR, 512], FP32, tag="td")
                nc.tensor.matmul(
                    ps_td[:, 0:w],
                    w_td_s[:],
                    ndeTl_s[:, lo : lo + w],
                    start=True,
                    stop=True,
                )
                nc.scalar.activation(td_allT[:, lo : lo + w], ps_td[:, 0:w], Silu)

            # --- merged S/G: sd build for group g, assembly for group g-1 ---
            # (stagger by one so the A-sum never waits on sdl's silu)
            for step in range(NG + 1):
                if step < NG:
                    g = step
                    t_ndeL = ndeL_tiles.pop(g)
                    ps = ps_sdl.tile([128, SCH * D_DIR], FP32, tag="sdl")
                    for c in range(SCH):
                        nc.tensor.matmul(
                            ps[:, c * D_DIR : (c + 1) * D_DIR],
                            t_ndeL[:, c * 128 : (c + 1) * 128],
                            w_sd_s[:],
                            start=True,
                            stop=True,
                        )
                    nc.scalar.activation(
                        sdl_all[:, g * 1024 : (g + 1) * 1024], ps[:], Silu
                    )
                    if g + 4 < NG:
                        ndeL_tiles[g + 4] = ndeL_pool.tile(
                            [D_DIR_IN + 1, SLOTS], FP16, tag="ndeL", name="ndeL_t"
                        )
                        nc.sync.dma_start(
                            ndeL_tiles[g + 4][:],
                            ndeL[:, (g + 4) * SLOTS : (g + 5) * SLOTS],
                        )
                    a_tiles[g] = a_pool.tile([128, SLOTS], FP16, tag="amat",
                                             name="a_t")
                    nc.sync.dma_start(
                        a_tiles[g][:], a_mat[:, g * SLOTS : (g + 1) * SLOTS]
                    )
                if step == 0:
                    continue
                g = step - 1
                t_a = a_tiles.pop(g)
                ps = ps_grp.tile([128, 512], FP32, tag="grp")
                # sender-dir sum: accumulate 16 slot chunks
                for c in range(SCH):
                    nc.tensor.matmul(
                        ps[:, 0:64],
                        t_a[:, c * 128 : (c + 1) * 128],
                        sdl_all[:, g * 1024 + c * 64 : g * 1024 + (c + 1) * 64],
                        start=(c == 0),
                        stop=(c == SCH - 1),
                    )
                # sender-atom sum via histogram
                nc.tensor.matmul(
                    ps[:, 64:192],
                    histT_s[:, g * 128 : (g + 1) * 128],
                    emb_s_s[:],
                    start=True,
                    stop=True,
                )
                # recv atom embedding (one-hot)
                nc.tensor.matmul(
                    ps[:, 192:320],
                    ohT_s[:, g * 128 : (g + 1) * 128],
                    emb_t_s[:],
                    start=True,
                    stop=True,
                )
                # dist + td column blocks -> node-major
                nc.tensor.transpose(
                    ps[:, 320:448], dist_acc[:, g * 128 : (g + 1) * 128], ident_s[:]
                )
                nc.tensor.transpose(
                    ps[:, 448:512],
                    td_allT[:, g * 128 : (g + 1) * 128],
                    ident_s[:D_DIR, :D_DIR],
                )
                out_t = out_pool.tile([128, 512], FP16)
                inv_g = scl_s[:, g : g + 1]
                cim_g = scl_s[:, NG + g : NG + g + 1]
                nc.vector.tensor_scalar_mul(out_t[:, 0:128], ps[:, 320:448], inv_g)
                nc.vector.tensor_scalar_mul(out_t[:, 128:192], ps[:, 0:64], inv_g)
                nc.vector.tensor_scalar_mul(out_t[:, 192:320], ps[:, 64:192], inv_g)
                nc.vector.tensor_scalar_mul(out_t[:, 320:384], ps[:, 448:512], cim_g)
                nc.vector.tensor_scalar_mul(out_t[:, 384:512], ps[:, 192:320], cim_g)
                rows = min(128, NLOC - g * 128)
                nc.sync.dma_start(
                    out_d[g * 128 : g * 128 + rows, :], out_t[:rows, :]
                )

    nc.compile()
    return nc


def _prep_core(c, atomic, nde, ede, nbr, mask):
    f32 = np.float32
    f16 = np.float16
    lo, hi = c * NLOC, (c + 1) * NLOC
    a_loc = atomic[lo:hi]
    nde_loc = nde[lo:hi]
    ede_loc = ede[lo:hi]
    nbr_loc = nbr[lo:hi]
    mask_loc = mask[lo:hi]

    # dist input: premask, pad, fp16, k-plane-major [128 fin, nb, k, 128 c]
    em = ede_loc * mask_loc[:, :, None].astype(f32)
    em_pad = np.zeros((NPAD, K, D_DIST_IN), dtype=f32)
    em_pad[:NLOC] = em
    edeT = np.ascontiguousarray(
        em_pad.reshape(NB, 128, K, D_DIST_IN).transpose(3, 0, 2, 1)
        .reshape(D_DIST_IN, NB * K * 128)
    ).astype(f16)

    # deduped neighbor slots per group
    mn = np.full((NPAD, K), -1, dtype=np.int64)
    mn[:NLOC] = np.where(mask_loc, nbr_loc, -1)
    a_all = np.zeros((NG, SLOTS, 128), dtype=f16)
    ndeL_h = np.zeros((D_DIR_IN + 1, NG * SLOTS), dtype=f16)
    ndeL_h[D_DIR_IN, :] = 1.0
    for g in range(NG):
        blk = mn[g * 128 : (g + 1) * 128]
        pp, kk = np.nonzero(blk >= 0)
        ids = blk[pp, kk]
        uniq, inv = np.unique(ids, return_inverse=True)
        u = uniq.shape[0]
        assert u <= SLOTS, f"group {g}: {u} unique neighbors > {SLOTS}"
        cnts = np.zeros((SLOTS, 128), dtype=f32)
        np.add.at(cnts, (inv, pp), 1.0)
        a_all[g] = cnts.astype(f16)
        ndeL_h[:D_DIR_IN, g * SLOTS : g * SLOTS + u] = nde[uniq].T.astype(f16)
    # A layout: [128 slot-in-chunk, (g, c, dst)]
    a_mat = np.ascontiguousarray(
        a_all.reshape(NG, SCH, 128, 128).transpose(2, 0, 1, 3)
        .reshape(128, NG * SLOTS)
    )

    # histogram of neighbor atom ids per dst node: histT[e, n]
    hg = np.zeros((NPAD, 128), dtype=f32)
    pp, kk = np.nonzero(mn >= 0)
    np.add.at(hg, (pp, atomic[mn[pp, kk]]), 1.0)
    histT_h = np.ascontiguousarray(hg.T).astype(f16)

    # one-hot of local node atom ids: ohT[e, n]
    ohT_h = np.zeros((128, NPAD), dtype=f16)
    ohT_h[a_loc.astype(np.int64), np.arange(NLOC)] = 1.0

    ndeTl = np.zeros((D_DIR_IN + 1, NPAD), dtype=f32)
    ndeTl[:D_DIR_IN, :NLOC] = nde_loc.T
    ndeTl[D_DIR_IN, :] = 1.0

    cnt = np.zeros((NPAD,), dtype=f32)
    cnt[:NLOC] = mask_loc.astype(f32).sum(1)
    inv_v = 1.0 / (cnt + 1e-5)
    cim_v = cnt * inv_v
    scl_h = np.zeros((128, 2 * NG), dtype=f32)
    scl_h[:, :NG] = inv_v.reshape(NG, 128).T
    scl_h[:, NG:] = cim_v.reshape(NG, 128).T

    return {
        "edeT": edeT,
        "a_mat": a_mat,
        "ndeL": np.ascontiguousarray(ndeL_h),
        "histT": histT_h,
        "ohT": np.ascontiguousarray(ohT_h),
        "ndeTl": np.ascontiguousarray(ndeTl),
        "scl": np.ascontiguousarray(scl_h),
    }


def _prepare_all(inputs):
    f32 = np.float32
    f16 = np.float16
    atomic = np.asarray(inputs["atomic_numbers"]).astype(np.int64)
    nde = np.asarray(inputs["node_direction_expansion"]).astype(f32)
    ede = np.asarray(inputs["edge_distance_expansion"]).astype(f32)
    nbr = np.asarray(inputs["neighbor_list"]).astype(np.int64)
    mask = np.asarray(inputs["neighbor_mask"]).astype(bool)
    emb_s = np.asarray(inputs["src_atom_emb"]).astype(f32)
    emb_t = np.asarray(inputs["tgt_atom_emb"]).astype(f32)
    w_sd = np.asarray(inputs["src_dir_W"]).astype(f32)
    b_sd = np.asarray(inputs["src_dir_b"]).astype(f32)
    w_td = np.asarray(inputs["tgt_dir_W"]).astype(f32)
    b_td = np.asarray(inputs["tgt_dir_b"]).astype(f32)
    w_di = np.asarray(inputs["dist_W"]).astype(f32)
    b_di = np.asarray(inputs["dist_b"]).astype(f32)
    assert np.all(b_di == 0.0), "nonzero dist_b not supported"

    wh = w_di.astype(f16)
    wl = (w_di - wh.astype(f32)).astype(f16)
    wdist2 = np.ascontiguousarray(np.concatenate([wh, wl], axis=1))
    emb_s_pad = np.zeros((128, D_ATOM), dtype=f16)
    emb_s_pad[:NUM_ELEM] = emb_s.astype(f16)
    emb_t_pad = np.zeros((128, D_ATOM), dtype=f16)
    emb_t_pad[:NUM_ELEM] = emb_t.astype(f16)

    shared = {
        "wdist2": wdist2,
        "w_sd": np.ascontiguousarray(np.vstack([w_sd, b_sd[None, :]]).astype(f16)),
        "w_td": np.ascontiguousarray(np.vstack([w_td, b_td[None, :]])),
        "emb_s": emb_s_pad,
        "emb_t": emb_t_pad,
        "ident": np.ascontiguousarray(np.eye(128, dtype=f32)),
    }

    in_maps = []
    for c in range(N_CORES):
        m = _prep_core(c, atomic, nde, ede, nbr, mask)
        m.update(shared)
        in_maps.append(m)
    return in_maps


def _run(inputs, trace=False, **spmd_kwargs):
    key = "prog"
    if key not in _CACHED:
        _CACHED[key] = _build_program()
    nc = _CACHED[key]

    in_maps = _prepare_all(inputs)
    res = run_bass_kernel_spmd(
        nc, in_maps, list(range(N_CORES)), trace=trace, **spmd_kwargs
    )
    out = np.concatenate([res.results[c]["out"] for c in range(N_CORES)], axis=0)
    return out.astype(np.float32), res


def kernel(**inputs):
    out, _ = _run(inputs, trace=False)
    return out


# revision 16
# speedup vs baseline: 1.1672x; 1.1672x over previous
"""Trainium2 Bass kernel for a GNN message-passing layer.

Reference computation (per node n, neighbors k=0..31):
  sa = src_atom_emb[atomic]            [N,128]
  ta = tgt_atom_emb[atomic]            [N,128]
  sd = silu(nde @ src_dir_W + b)       [N,64]
  td = silu(nde @ tgt_dir_W + b)       [N,64]
  edist = silu(ede @ dist_W + b)       [N,K,128]
  feat  = [edist | sd[nbr] | sa[nbr] | td | ta]   [N,K,512]
  out   = sum_k(mask*feat) / (sum_k mask + 1e-5)  [N,512]

Strategy (8 cores, nodes sharded 1250/core, SPMD, no collectives, and
no SWDGE gather -- the per-row descriptor cost dominated the previous
design):
  - dist branch: host premasks ede (silu(0)=0 for 0/1 masks), casts to
    fp16, and lays it out k-plane-major [128 fin, nb, k, 128 nodes].
    PE streams it against dist_W split into fp16 hi+lo halves (two
    accumulating matmuls recover near-fp32 precision; plain bf16/fp16
    weights fail the 2e-2 rel bar because ~2% of outputs sit below the
    rel-err denominator floor).  SiLU on ACT in fp32, then the k-sum is
    5 fully-contiguous pairwise tree adds on DVE (the strided
    tensor_reduce form runs ~5x slower).
  - sender-dir sums: the host dedupes each 128-node group's neighbor
    set (max 1943 unique, padded to 2048 slots), ships the gathered nde
    rows (fp16) plus an fp16 multiplicity matrix A[slot, dst].  sd is
    built on-PE (fp16, exact-count A), and the neighbor sum is 16
    accumulating [128x128]x[128,64] matmuls per group.
  - sender-atom sums: only 100 distinct embedding rows exist, so the
    host ships a per-node histogram over atom ids (fp16, exact counts)
    and the sum is one matmul per group against the fp16 embedding.
  - recv features: td via fp32 matmuls (per-node values get no
    mask-mean averaging, so they need full precision) + PE transpose;
    ta via one-hot matmul.  Scales cnt/(cnt+eps), 1/(cnt+eps) come
    precomputed from host.
"""

import os
import sys

import numpy as np

sys.path.insert(0, "/opt/trn_rl_repo")

import concourse.bacc as bacc  # noqa: E402
import concourse.bass as bass  # noqa: E402,F401
import concourse.mybir as mybir  # noqa: E402
import concourse.tile as tile  # noqa: E402
from concourse.bass_utils import run_bass_kernel_spmd  # noqa: E402

# Problem shape (hardcoded; harness always uses these).
N_CORES = 8
N = 10000
K = 32
NLOC = N // N_CORES          # 1250 nodes per core
NPAD = 1280                  # padded to 10 groups of 128
NG = NPAD // 128             # 10 node groups
NB = NG                      # dist branch node blocks (128 nodes each)
D_DIR_IN = 10
D_DIR = 64
D_ATOM = 128
D_DIST_IN = 128
D_DIST = 128
NUM_ELEM = 100
SLOTS = 2048                 # deduped neighbor slots per group
SCH = SLOTS // 128           # 16 slot chunks per group
FP32 = mybir.dt.float32
FP16 = mybir.dt.float16

_CACHED = {}


def _build_program():
    nc = bacc.Bacc(
        "TRN2",
        target_bir_lowering=False,
        debug=False,
        enable_asserts=False,
        num_devices=N_CORES,
    )

    # per-core inputs
    edeT = nc.dram_tensor("edeT", [128, NB * K * 128], FP16, kind="ExternalInput")
    a_mat = nc.dram_tensor("a_mat", [128, NG * SLOTS], FP16, kind="ExternalInput")
    ndeL = nc.dram_tensor("ndeL", [D_DIR_IN + 1, NG * SLOTS], FP16,
                          kind="ExternalInput")
    histT = nc.dram_tensor("histT", [128, NPAD], FP16, kind="ExternalInput")
    ohT = nc.dram_tensor("ohT", [128, NPAD], FP16, kind="ExternalInput")
    ndeTl = nc.dram_tensor("ndeTl", [D_DIR_IN + 1, NPAD], FP32, kind="ExternalInput")
    scl = nc.dram_tensor("scl", [128, 2 * NG], FP32, kind="ExternalInput")
    # shared (replicated) parameters
    wdist2 = nc.dram_tensor("wdist2", [D_DIST_IN, 2 * D_DIST], FP16,
                            kind="ExternalInput")
    w_sd = nc.dram_tensor("w_sd", [D_DIR_IN + 1, D_DIR], FP16, kind="ExternalInput")
    w_td = nc.dram_tensor("w_td", [D_DIR_IN + 1, D_DIR], FP32, kind="ExternalInput")
    emb_s = nc.dram_tensor("emb_s", [128, D_ATOM], FP16, kind="ExternalInput")
    emb_t = nc.dram_tensor("emb_t", [128, D_ATOM], FP16, kind="ExternalInput")
    ident = nc.dram_tensor("ident", [128, 128], FP32, kind="ExternalInput")

    # fp16 output: rounding error is proportional to the value (<=2^-11 rel),
    # so it can never violate the floored rel-err metric; halves output DMA.
    out_d = nc.dram_tensor("out", [NLOC, 512], FP16, kind="ExternalOutput")

    Silu = mybir.ActivationFunctionType.Silu

    with tile.TileContext(nc) as tc:
        from contextlib import ExitStack

        with ExitStack() as ctx:
            const = ctx.enter_context(tc.tile_pool(name="const", bufs=1))
            ede_pool = ctx.enter_context(tc.tile_pool(name="ede_pool", bufs=3))
            silu_pool = ctx.enter_context(tc.tile_pool(name="silu_pool", bufs=2))
            t1_pool = ctx.enter_context(tc.tile_pool(name="t1_pool", bufs=2))
            t2_pool = ctx.enter_context(tc.tile_pool(name="t2_pool", bufs=2))
            t34_pool = ctx.enter_context(tc.tile_pool(name="t34_pool", bufs=2))
            ndeL_pool = ctx.enter_context(tc.tile_pool(name="ndeL_pool", bufs=4))
            a_pool = ctx.enter_context(tc.tile_pool(name="a_pool", bufs=6))
            out_pool = ctx.enter_context(tc.tile_pool(name="out_pool", bufs=2))
            ps_grp = ctx.enter_context(
                tc.tile_pool(name="ps_grp", bufs=2, space="PSUM")
            )

            # --- kick weights first, then the ede stream, then small consts ---
            wdist2_s = const.tile([D_DIST_IN, 2 * D_DIST], FP16)
            nc.sync.dma_start(wdist2_s[:], wdist2[:, :])
            ede_tiles = {}
            for j in (0, 1, 2):
                ede_tiles[j] = ede_pool.tile([128, K * 128], FP16, tag="ede",
                                             name="ede_t")
                nc.sync.dma_start(
                    ede_tiles[j][:], edeT[:, j * K * 128 : (j + 1) * K * 128]
                )
            w_sd_s = const.tile([D_DIR_IN + 1, D_DIR], FP16)
            nc.sync.dma_start(w_sd_s[:], w_sd[:, :])
            w_td_s = const.tile([D_DIR_IN + 1, D_DIR], FP32)
            nc.sync.dma_start(w_td_s[:], w_td[:, :])
            emb_s_s = const.tile([128, D_ATOM], FP16)
            nc.sync.dma_start(emb_s_s[:], emb_s[:, :])
            emb_t_s = const.tile([128, D_ATOM], FP16)
            nc.sync.dma_start(emb_t_s[:], emb_t[:, :])
            ident_s = const.tile([128, 128], FP32)
            nc.sync.dma_start(ident_s[:], ident[:, :])
            scl_s = const.tile([128, 2 * NG], FP32)
            nc.sync.dma_start(scl_s[:], scl[:, :])
            histT_s = const.tile([128, NPAD], FP16)
            nc.sync.dma_start(histT_s[:], histT[:, :])
            ohT_s = const.tile([128, NPAD], FP16)
            nc.sync.dma_start(ohT_s[:], ohT[:, :])
            ndeTl_s = const.tile([D_DIR_IN + 1, NPAD], FP32)
            nc.sync.dma_start(ndeTl_s[:], ndeTl[:, :])
            ndeL_tiles = {}
            for g in range(4):
                ndeL_tiles[g] = ndeL_pool.tile([D_DIR_IN + 1, SLOTS], FP16,
                                               tag="ndeL", name="ndeL_t")
                nc.sync.dma_start(
                    ndeL_tiles[g][:], ndeL[:, g * SLOTS : (g + 1) * SLOTS]
                )
            a_tiles = {}

            dist_acc = const.tile([128, NPAD], FP32)
            sdl_all = const.tile([128, NG * SLOTS // 2], FP16)  # [128, g, 16, 64]
            td_allT = const.tile([D_DIR, NPAD], FP32)

            # --- Phase D: dist branch (k-plane matmuls + silu + tree sums) ---
            with ExitStack() as pd_ctx:
                ps_dist = pd_ctx.enter_context(
                    tc.tile_pool(name="ps_dist", bufs=2, space="PSUM")
                )
                for nb in range(NB):
                    t_ede = ede_tiles.pop(nb)
                    s_tile = silu_pool.tile([128, K * 128], FP32, tag="silu")
                    for q in range(4):  # 4 psum tiles x 1024 cols
                        pd = ps_dist.tile([128, 1024], FP32, tag="pd")
                        for r in range(2):  # two 512-col regions
                            lo = q * 1024 + r * 512
                            nc.tensor.matmul(
                                pd[:, r * 512 : (r + 1) * 512],
                                wdist2_s[:, 0:D_DIST],
                                t_ede[:, lo : lo + 512],
                                start=True,
                                stop=False,
                            )
                            nc.tensor.matmul(
                                pd[:, r * 512 : (r + 1) * 512],
                                wdist2_s[:, D_DIST : 2 * D_DIST],
                                t_ede[:, lo : lo + 512],
                                start=False,
                                stop=True,
                            )
                        nc.scalar.activation(
                            s_tile[:, q * 1024 : (q + 1) * 1024], pd[:], Silu
                        )
                    if nb + 3 < NB:
                        ede_tiles[nb + 3] = ede_pool.tile(
                            [128, K * 128], FP16, tag="ede", name="ede_t"
                        )
                        nc.sync.dma_start(
                            ede_tiles[nb + 3][:],
                            edeT[:, (nb + 3) * K * 128 : (nb + 4) * K * 128],
                        )
                    # pairwise tree sum over the 32 k-planes (all contiguous).
                    # Pool (slow sw add) takes half of round 1; DVE the rest.
                    t1 = t1_pool.tile([128, 2048], FP32, tag="t1")
                    nc.gpsimd.tensor_add(
                        t1[:, 0:1024], s_tile[:, 0:1024], s_tile[:, 2048:3072]
                    )
                    nc.vector.tensor_add(
                        t1[:, 1024:2048], s_tile[:, 1024:2048], s_tile[:, 3072:4096]
                    )
                    t2 = t2_pool.tile([128, 1024], FP32, tag="t2")
                    nc.vector.tensor_add(t2[:], t1[:, 0:1024], t1[:, 1024:2048])
                    t3 = t34_pool.tile([128, 512], FP32, tag="t3")
                    nc.vector.tensor_add(t3[:], t2[:, 0:512], t2[:, 512:1024])
                    t4 = t34_pool.tile([128, 256], FP32, tag="t4")
                    nc.vector.tensor_add(t4[:], t3[:, 0:256], t3[:, 256:512])
                    nc.vector.tensor_add(
                        dist_acc[:, nb * 128 : (nb + 1) * 128],
                        t4[:, 0:128],
                        t4[:, 128:256],
                    )

            # --- Phase S: per-group sd over deduped slots + td batch ---
            with ExitStack() as ps_ctx:
                ps_sdl = ps_ctx.enter_context(
                    tc.tile_pool(name="ps_sdl", bufs=2, space="PSUM")
                )
                for g in range(NG):
                    t_ndeL = ndeL_tiles.pop(g)
                    ps = ps_sdl.tile([128, SCH * D_DIR], FP32, tag="sdl")
                    for c in range(SCH):
                        nc.tensor.matmul(
                            ps[:, c * D_DIR : (c + 1) * D_DIR],
                            t_ndeL[:, c * 128 : (c + 1) * 128],
                            w_sd_s[:],
                            start=True,
                            stop=True,
                        )
                    nc.scalar.activation(
                        sdl_all[:, g * 1024 : (g + 1) * 1024], ps[:], Silu
                    )
                    if g + 4 < NG:
                        ndeL_tiles[g + 4] = ndeL_pool.tile(
                            [D_DIR_IN + 1, SLOTS], FP16, tag="ndeL", name="ndeL_t"
                        )
                        nc.sync.dma_start(
                            ndeL_tiles[g + 4][:],
                            ndeL[:, (g + 4) * SLOTS : (g + 5) * SLOTS],
                        )
                    if g <= 5:
                        a_tiles[g] = a_pool.tile([128, SLOTS], FP16, tag="amat",
                                                 name="a_t")
                        nc.sync.dma_start(
                            a_tiles[g][:], a_mat[:, g * SLOTS : (g + 1) * SLOTS]
                        )

                # td for all local nodes: [64, NPAD] feature-major (fp32)
                for i, (lo, w) in enumerate([(0, 512), (512, 512), (1024, 256)]):
                    ps_td = ps_sdl.tile([D_DIR, 512], FP32, tag="td")
                    nc.tensor.matmul(
                        ps_td[:, 0:w],
                        w_td_s[:],
                        ndeTl_s[:, lo : lo + w],
                        start=True,
                        stop=True,
                    )
                    nc.scalar.activation(
                        td_allT[:, lo : lo + w], ps_td[:, 0:w], Silu
                    )

            # --- Phase G: neighbor sums + recv features + output assembly ---
            for g in range(NG):
                t_a = a_tiles.pop(g)
                ps = ps_grp.tile([128, 512], FP32, tag="grp")
                # sender-dir sum: accumulate 16 slot chunks
                for c in range(SCH):
                    nc.tensor.matmul(
                        ps[:, 0:64],
                        t_a[:, c * 128 : (c + 1) * 128],
                        sdl_all[:, g * 1024 + c * 64 : g * 1024 + (c + 1) * 64],
                        start=(c == 0),
                        stop=(c == SCH - 1),
                    )
                # sender-atom sum via histogram
                nc.tensor.matmul(
                    ps[:, 64:192],
                    histT_s[:, g * 128 : (g + 1) * 128],
                    emb_s_s[:],
                    start=True,
                    stop=True,
                )
                # recv atom embedding (one-hot)
                nc.tensor.matmul(
                    ps[:, 192:320],
                    ohT_s[:, g * 128 : (g + 1) * 128],
                    emb_t_s[:],
                    start=True,
                    stop=True,
                )
                # dist + td column blocks -> node-major
                nc.tensor.transpose(
                    ps[:, 320:448], dist_acc[:, g * 128 : (g + 1) * 128], ident_s[:]
                )
                nc.tensor.transpose(
                    ps[:, 448:512],
                    td_allT[:, g * 128 : (g + 1) * 128],
                    ident_s[:D_DIR, :D_DIR],
                )
                if g + 6 < NG:
                    a_tiles[g + 6] = a_pool.tile([128, SLOTS], FP16, tag="amat",
                                                 name="a_t")
                    nc.sync.dma_start(
                        a_tiles[g + 6][:],
                        a_mat[:, (g + 6) * SLOTS : (g + 7) * SLOTS],
                    )
                out_t = out_pool.tile([128, 512], FP16)
                inv_g = scl_s[:, g : g + 1]
                cim_g = scl_s[:, NG + g : NG + g + 1]
                nc.vector.tensor_scalar_mul(out_t[:, 0:128], ps[:, 320:448], inv_g)
                nc.vector.tensor_scalar_mul(out_t[:, 128:192], ps[:, 0:64], inv_g)
                nc.vector.tensor_scalar_mul(out_t[:, 192:320], ps[:, 64:192], inv_g)
                nc.vector.tensor_scalar_mul(out_t[:, 320:384], ps[:, 448:512], cim_g)
                nc.vector.tensor_scalar_mul(out_t[:, 384:512], ps[:, 192:320], cim_g)
                rows = min(128, NLOC - g * 128)
                nc.sync.dma_start(
                    out_d[g * 128 : g * 128 + rows, :], out_t[:rows, :]
                )

    nc.compile()
    return nc


def _prep_core(c, atomic, nde, ede, nbr, mask):
    f32 = np.float32
    f16 = np.float16
    lo, hi = c * NLOC, (c + 1) * NLOC
    a_loc = atomic[lo:hi]
    nde_loc = nde[lo:hi]
    ede_loc = ede[lo:hi]
    nbr_loc = nbr[lo:hi]
    mask_loc = mask[lo:hi]

    # dist input: premask, pad, fp16, k-plane-major [128 fin, nb, k, 128 c]
    em = ede_loc * mask_loc[:, :, None].astype(f32)
    em_pad = np.zeros((NPAD, K, D_DIST_IN), dtype=f32)
    em_pad[:NLOC] = em
    edeT = np.ascontiguousarray(
        em_pad.reshape(NB, 128, K, D_DIST_IN).transpose(3, 0, 2, 1)
        .reshape(D_DIST_IN, NB * K * 128)
    ).astype(f16)

    # deduped neighbor slots per group
    mn = np.full((NPAD, K), -1, dtype=np.int64)
    mn[:NLOC] = np.where(mask_loc, nbr_loc, -1)
    a_all = np.zeros((NG, SLOTS, 128), dtype=f16)
    ndeL_h = np.zeros((D_DIR_IN + 1, NG * SLOTS), dtype=f16)
    ndeL_h[D_DIR_IN, :] = 1.0
    for g in range(NG):
        blk = mn[g * 128 : (g + 1) * 128]
        pp, kk = np.nonzero(blk >= 0)
        ids = blk[pp, kk]
        uniq, inv = np.unique(ids, return_inverse=True)
        u = uniq.shape[0]
        assert u <= SLOTS, f"group {g}: {u} unique neighbors > {SLOTS}"
        cnts = np.zeros((SLOTS, 128), dtype=f32)
        np.add.at(cnts, (inv, pp), 1.0)
        a_all[g] = cnts.astype(f16)
        ndeL_h[:D_DIR_IN, g * SLOTS : g * SLOTS + u] = nde[uniq].T.astype(f16)
    # A layout: [128 slot-in-chunk, (g, c, dst)]
    a_mat = np.ascontiguousarray(
        a_all.reshape(NG, SCH, 128, 128).transpose(2, 0, 1, 3)
        .reshape(128, NG * SLOTS)
    )

    # histogram of neighbor atom ids per dst node: histT[e, n]
    hg = np.zeros((NPAD, 128), dtype=f32)
    pp, kk = np.nonzero(mn >= 0)
    np.add.at(hg, (pp, atomic[mn[pp, kk]]), 1.0)
    histT_h = np.ascontiguousarray(hg.T).astype(f16)

    # one-hot of local node atom ids: ohT[e, n]
    ohT_h = np.zeros((128, NPAD), dtype=f16)
    ohT_h[a_loc.astype(np.int64), np.arange(NLOC)] = 1.0

    ndeTl = np.zeros((D_DIR_IN + 1, NPAD), dtype=f32)
    ndeTl[:D_DIR_IN, :NLOC] = nde_loc.T
    ndeTl[D_DIR_IN, :] = 1.0

    cnt = np.zeros((NPAD,), dtype=f32)
    cnt[:NLOC] = mask_loc.astype(f32).sum(1)
    inv_v = 1.0 / (cnt + 1e-5)
    cim_v = cnt * inv_v
    scl_h = np.zeros((128, 2 * NG), dtype=f32)
    scl_h[:, :NG] = inv_v.reshape(NG, 128).T
    scl_h[:, NG:] = cim_v.reshape(NG, 128).T

    return {
        "edeT": edeT,
        "a_mat": a_mat,
        "ndeL": np.ascontiguousarray(ndeL_h),
        "histT": histT_h,
        "ohT": np.ascontiguousarray(ohT_h),
        "ndeTl": np.ascontiguousarray(ndeTl),
        "scl": np.ascontiguousarray(scl_h),
    }


def _prepare_all(inputs):
    f32 = np.float32
    f16 = np.float16
    atomic = np.asarray(inputs["atomic_numbers"]).astype(np.int64)
    nde = np.asarray(inputs["node_direction_expansion"]).astype(f32)
    ede = np.asarray(inputs["edge_distance_expansion"]).astype(f32)
    nbr = np.asarray(inputs["neighbor_list"]).astype(np.int64)
    mask = np.asarray(inputs["neighbor_mask"]).astype(bool)
    emb_s = np.asarray(inputs["src_atom_emb"]).astype(f32)
    emb_t = np.asarray(inputs["tgt_atom_emb"]).astype(f32)
    w_sd = np.asarray(inputs["src_dir_W"]).astype(f32)
    b_sd = np.asarray(inputs["src_dir_b"]).astype(f32)
    w_td = np.asarray(inputs["tgt_dir_W"]).astype(f32)
    b_td = np.asarray(inputs["tgt_dir_b"]).astype(f32)
    w_di = np.asarray(inputs["dist_W"]).astype(f32)
    b_di = np.asarray(inputs["dist_b"]).astype(f32)
    assert np.all(b_di == 0.0), "nonzero dist_b not supported"

    wh = w_di.astype(f16)
    wl = (w_di - wh.astype(f32)).astype(f16)
    wdist2 = np.ascontiguousarray(np.concatenate([wh, wl], axis=1))
    emb_s_pad = np.zeros((128, D_ATOM), dtype=f16)
    emb_s_pad[:NUM_ELEM] = emb_s.astype(f16)
    emb_t_pad = np.zeros((128, D_ATOM), dtype=f16)
    emb_t_pad[:NUM_ELEM] = emb_t.astype(f16)

    shared = {
        "wdist2": wdist2,
        "w_sd": np.ascontiguousarray(np.vstack([w_sd, b_sd[None, :]]).astype(f16)),
        "w_td": np.ascontiguousarray(np.vstack([w_td, b_td[None, :]])),
        "emb_s": emb_s_pad,
        "emb_t": emb_t_pad,
        "ident": np.ascontiguousarray(np.eye(128, dtype=f32)),
    }

    in_maps = []
    for c in range(N_CORES):
        m = _prep_core(c, atomic, nde, ede, nbr, mask)
        m.update(shared)
        in_maps.append(m)
    return in_maps


def _run(inputs, trace=False, **spmd_kwargs):
    key = "prog"
    if key not in _CACHED:
        _CACHED[key] = _build_program()
    nc = _CACHED[key]

    in_maps = _prepare_all(inputs)
    res = run_bass_kernel_spmd(
        nc, in_maps, list(range(N_CORES)), trace=trace, **spmd_kwargs
    )
    out = np.concatenate([res.results[c]["out"] for c in range(N_CORES)], axis=0)
    return out.astype(np.float32), res


def kernel(**inputs):
    out, _ = _run(inputs, trace=False)
    return out
